# revision 8
# baseline (speedup 1.0000x reference)
"""Trainium2 Bass kernel for nn_Deep_AD_F_58213986730479 (dense_cnn).

Math (per iteration t of 3):
    feats = 4 one-pixel zero-padded shifts (N,S,W,E) of x        [n,4,h,w]
    d     = conv3x3(feats, W[t]) + b[t]                          [n,4,h,w]
    x    -= sum_k d_k * exp(-d_k^2) / 4

Implementation (v2, fp8 DoubleRow):
  - Pure data parallel: batch 32 -> 8 cores x 4 images.
  - The shift+conv composes into a 21-tap stencil: 5 column-banded matrices
    B_Dx (Dx=-2..2) applied at column shifts. fp8e4 DoubleRow matmuls pair
    two (lhsT, shifted-rhs) slots per instruction at 0.5 cyc/row, so the 5
    shifts fit in 3 matmuls per (k, tile): (-2,-1), (0,+1), (+2, zero).
    Guard columns (2 left, 6 right) on the fp8 x copy let every pair run the
    full 512-col output range; the per-channel bias is folded into weight row
    127 of the Dx=0 slot against an all-ones row 127 of the fp8 x tile.
  - Column-edge phantom corrections (corrL/corrR) are 1-col fp8 matmuls;
    row-edge corrections are baked into top/bot class variants of B_{-1,0,1}.
  - d for all 4 channels lands in one [128, 2048] PSUM tile (4 banks,
    double-buffered): ONE batched Derivative_Erf activation per step
    (e = 2/sqrt(pi) exp(-d^2), bias rides in the matmul), gated = d*e split
    DVE/GpSimd, bf16 4x-mode sums + masked update on DVE.
  - x is stored bf16 between iterations (update STT writes it), converted to
    fp8 for the next iteration's matmuls on GpSimd/Scalar; the last
    iteration's update writes f32 straight into the staging tile for DMA out.
  - 512 rows -> 5 tiles owning 103/103/103/103/100 rows, 6-row halo each
    side (valid region shrinks 2 rows/iter; partition 127 reserved for the
    ones row, partitions 6.. hold owned rows).
"""
import sys

sys.path.insert(0, "/opt/trn_rl_repo")

import math
import numpy as np

import concourse.bass as bass
import concourse.bacc as bacc
import concourse.mybir as mybir
from concourse.ap import AP
from concourse.tile import TileContext
from concourse.bass_utils import run_bass_kernel_spmd

F32 = mybir.dt.float32
BF16 = mybir.dt.bfloat16
FP8 = mybir.dt.float8e4
AF = mybir.ActivationFunctionType
ALU = mybir.AluOpType
DR = mybir.MatmulPerfMode.DoubleRow

NCORES = 8
IMGS = 4
H = W_IMG = 512
T_ITERS = 3
KCH = 4
NTILES = 5
OWN = [103, 103, 103, 103, 100]      # owned rows per tile
OSTART = [0, 103, 206, 309, 412]     # first owned image row
RSTART = [-6, 97, 200, 303, 406]     # image row at partition 0
PLO = [6, 0, 0, 0, 0]                # first loaded partition
PHI = [128, 128, 128, 128, 106]      # end of loaded partitions
MLO = [6, 0, 0, 0, 0]                # update-mask range
MHI = [115, 115, 115, 115, 106]
OWN_P0 = 6                           # owned rows start here in every tile
ONES_P = 127                         # all-ones row for the bias trick
C_UPD = math.sqrt(math.pi) / 8.0     # 1/4 * sqrt(pi)/2 (Derivative_Erf scale)
XQW = 520                            # fp8 x tile width: 2 guard + 512 + 6 guard
GL = 2                               # left guard cols

# feats channel order in reference: N, S, W, E
OY = [-1, 1, 0, 0]
OX = [0, 0, -1, 1]

WTILE_COLS = 2048   # per-(t,k) weight tile: 3*P1(256) 3*P2(256) P3(256) corrL/R
DVE_COLS = 1152     # gated-multiply split point (rest on GpSimd)


def _composite_taps(Wc):
    taps = np.zeros((T_ITERS, KCH, 5, 5), np.float64)
    for t in range(T_ITERS):
        for k in range(KCH):
            for i in range(4):
                for dy in (-1, 0, 1):
                    for dx in (-1, 0, 1):
                        taps[t, k, dy + OY[i] + 2, dx + OX[i] + 2] += Wc[
                            t, k, i, dy + 1, dx + 1
                        ]
    return taps


def _band(vals_by_dy):
    B = np.zeros((128, 128), np.float64)
    for dy, v in vals_by_dy.items():
        B += v * np.eye(128, k=-dy)
    return B


def _build_wq(Wc, bc):
    """fp8 weight image [128, 12*2048]: per (t,k) the pair/corr matrices."""
    taps = _composite_taps(Wc)
    out = np.zeros((128, T_ITERS * KCH * WTILE_COLS), np.float64)
    for t in range(T_ITERS):
        for k in range(KCH):
            base = (t * KCH + k) * WTILE_COLS
            per_dx = {
                Dx: _band({Dy: taps[t, k, Dy + 2, Dx + 2] for Dy in range(-2, 3)})
                for Dx in (-2, -1, 0, 1, 2)
            }
            variants = {}
            for cls, fix in (("top", 0), ("mid", None), ("bot", 1)):
                v = {}
                for Dx in (-1, 0, 1):
                    B = per_dx[Dx].copy()
                    if fix == 0:
                        # image row 0 at partition 6: remove south-ch phantom
                        B[OWN_P0, OWN_P0] -= Wc[t, k, 1, 0, Dx + 1]
                    elif fix == 1:
                        # image row 511 at partition 105: remove north-ch phantom
                        p = 105
                        B[p, p] -= Wc[t, k, 0, 2, Dx + 1]
                    v[Dx] = B
                variants[cls] = v
            for ci, cls in enumerate(("top", "mid", "bot")):
                v = variants[cls]
                # P1 = (B_{-2}, B_{-1,cls})
                off = base + ci * 256
                out[:, off : off + 128] = per_dx[-2]
                out[:, off + 128 : off + 256] = v[-1]
                # P2 = (B_{0,cls} + bias row, B_{+1,cls})
                off = base + 768 + ci * 256
                B0 = v[0].copy()
                B0[ONES_P, :] = bc[t, k]
                out[:, off : off + 128] = B0
                out[:, off + 128 : off + 256] = v[1]
            # P3 = (B_{+2}, zeros)
            off = base + 1536
            out[:, off : off + 128] = per_dx[2]
            # corrections: east-ch phantom at col 0, west-ch at col 511
            out[:, base + 1792 : base + 1920] = _band(
                {dy: -Wc[t, k, 3, dy + 1, 0] for dy in (-1, 0, 1)}
            )
            out[:, base + 1920 : base + 2048] = _band(
                {dy: -Wc[t, k, 2, dy + 1, 2] for dy in (-1, 0, 1)}
            )
    npdt = mybir.dt.np(FP8)
    return np.ascontiguousarray(out.astype(np.float32)).astype(npdt)


def _build_aux():
    m = np.zeros((128, 8), np.float32)
    for j in range(NTILES):
        m[MLO[j] : MHI[j], j] = -C_UPD
    return m


def _build_nc():
    nc = bacc.Bacc(None, target_bir_lowering=False)
    xs = nc.declare_dram_parameter("xs", [IMGS, H, W_IMG], F32, isOutput=False)
    wq = nc.declare_dram_parameter(
        "wq", [128, T_ITERS * KCH * WTILE_COLS], FP8, isOutput=False
    )
    aux = nc.declare_dram_parameter("aux", [128, 8], F32, isOutput=False)
    yo = nc.declare_dram_parameter("out", [IMGS, H, W_IMG], F32, isOutput=True)

    with TileContext(nc) as tc:
        with (
            tc.tile_pool(name="wts", bufs=1) as wp,
            tc.tile_pool(name="xdata", bufs=1) as xp,
            tc.tile_pool(name="work", bufs=3) as sp,
            tc.tile_pool(name="ps", bufs=2, space="PSUM") as pp,
        ):
            auxt = wp.tile([128, 8], F32, tag="auxt")
            nc.sync.dma_start(out=auxt[:], in_=aux[:])
            ones_t = wp.tile([128, W_IMG], FP8, tag="ones")
            nc.vector.memset(ones_t[:], 1.0)

            wt = {}
            for t in range(T_ITERS):
                for k in range(KCH):
                    wt[t, k] = wp.tile(
                        [128, WTILE_COLS], FP8, tag=f"wt{t}_{k}", name=f"wt{t}_{k}"
                    )

            def load_wt(t, k):
                off = (t * KCH + k) * WTILE_COLS
                nc.sync.dma_start(
                    out=wt[t, k][:], in_=wq[:, off : off + WTILE_COLS]
                )

            stage, xb, xq = {}, {}, {}
            conv_eng = [nc.scalar, nc.gpsimd, nc.vector]
            for im in range(IMGS):
                for j in range(NTILES):
                    st = xp.tile([128, W_IMG], F32, tag=f"st{im}_{j}", name=f"st{im}_{j}")
                    stage[im, j] = st
                    if PLO[j] > 0:
                        nc.vector.memset(st[0 : PLO[j], :], 0.0)
                    if PHI[j] < 128:
                        # engine APs must start at a multiple of 32; the DMA
                        # below overwrites the loaded sub-range afterwards
                        p0 = (PHI[j] // 32) * 32
                        nc.vector.memset(st[p0:128, :], 0.0)
                    nc.sync.dma_start(
                        out=st[PLO[j] : PHI[j], :],
                        in_=xs[im, RSTART[j] + PLO[j] : RSTART[j] + PHI[j], :],
                    )
                    if im == 0 and j < KCH:
                        load_wt(0, j)
                if 1 <= im < T_ITERS:
                    for k in range(KCH):
                        load_wt(im, k)

            for im in range(IMGS):
                for j in range(NTILES):
                    q = xp.tile([128, XQW], FP8, tag=f"xq{im}_{j}", name=f"xq{im}_{j}")
                    xq[im, j] = q
                    nc.vector.memset(q[:], 0.0)
                    nc.sync.dma_start(
                        out=q[ONES_P : ONES_P + 1, GL : GL + W_IMG],
                        in_=ones_t[0:1, :],
                    )
                    xb[im, j] = xp.tile(
                        [128, W_IMG], BF16, tag=f"xb{im}_{j}", name=f"xb{im}_{j}"
                    )
                    # initial f32 -> fp8 conversion, round-robined over engines
                    eng = conv_eng[(im * NTILES + j) % 3]
                    if eng is nc.scalar:
                        eng.copy(q[0:ONES_P, GL : GL + W_IMG], stage[im, j][0:ONES_P, :])
                    else:
                        eng.tensor_copy(
                            out=q[0:ONES_P, GL : GL + W_IMG],
                            in_=stage[im, j][0:ONES_P, :],
                        )

            rhs_dims = [[XQW, 128], [1, 2], [1, W_IMG]]
            lhs_dims = [[WTILE_COLS, 128], [128, 2], [1, 128]]

            for t in range(T_ITERS):
                for im in range(IMGS):
                    for j in range(NTILES):
                        cls = 0 if j == 0 else (2 if j == NTILES - 1 else 1)
                        q = xq[im, j]
                        qh, qoff = q[:].tensor, q[:].offset
                        d = pp.tile([128, KCH * W_IMG], F32, tag="d", name="d")
                        for k in range(KCH):
                            w = wt[t, k]
                            wh, woff = w[:].tensor, w[:].offset
                            ob = k * W_IMG
                            for pi, (wcol, xcol) in enumerate(
                                ((cls * 256, 0), (768 + cls * 256, 2), (1536, 4))
                            ):
                                nc.tensor.matmul(
                                    d[:, ob : ob + W_IMG],
                                    AP(wh, woff + wcol, lhs_dims),
                                    AP(qh, qoff + xcol, rhs_dims),
                                    start=(pi == 0),
                                    stop=False,
                                    perf_mode=DR,
                                )
                            nc.tensor.matmul(
                                d[:, ob : ob + 1],
                                w[:, 1792:1920],
                                q[:, GL : GL + 1],
                                start=False,
                                stop=False,
                            )
                            nc.tensor.matmul(
                                d[:, ob + W_IMG - 1 : ob + W_IMG],
                                w[:, 1920:2048],
                                q[:, GL + W_IMG - 1 : GL + W_IMG],
                                start=False,
                                stop=True,
                            )
                        e = sp.tile([128, KCH * W_IMG], BF16, tag="e")
                        nc.scalar.activation(e[:], d[:], AF.Derivative_Erf)
                        # GpSimd cannot touch PSUM, so DVE owns the gated
                        # multiply and GpSimd the bf16 sum tree
                        g = sp.tile([128, KCH * W_IMG], BF16, tag="g")
                        nc.vector.tensor_tensor(
                            out=g[:], in0=d[:], in1=e[:], op=ALU.mult
                        )
                        s01 = sp.tile([128, 2 * W_IMG], BF16, tag="s01")
                        nc.gpsimd.tensor_tensor(
                            out=s01[:],
                            in0=g[:, 0 : 2 * W_IMG],
                            in1=g[:, 2 * W_IMG :],
                            op=ALU.add,
                        )
                        stot = sp.tile([128, W_IMG], BF16, tag="stot")
                        nc.vector.tensor_tensor(
                            out=stot[:],
                            in0=s01[:, 0:W_IMG],
                            in1=s01[:, W_IMG:],
                            op=ALU.add,
                        )
                        mask = auxt[:, j : j + 1]
                        if t == 0:
                            upd_out, upd_in = xb[im, j][:], stage[im, j][:]
                        elif t == T_ITERS - 1:
                            upd_out, upd_in = stage[im, j][:], xb[im, j][:]
                        else:
                            upd_out = upd_in = xb[im, j][:]
                        nc.vector.scalar_tensor_tensor(
                            out=upd_out,
                            in0=stot[:],
                            scalar=mask,
                            in1=upd_in,
                            op0=ALU.mult,
                            op1=ALU.add,
                        )
                        if t == T_ITERS - 1:
                            nc.sync.dma_start(
                                out=yo[im, OSTART[j] : OSTART[j] + OWN[j], :],
                                in_=stage[im, j][OWN_P0 : OWN_P0 + OWN[j], :],
                            )
                        else:
                            nc.scalar.copy(
                                xq[im, j][0:ONES_P, GL : GL + W_IMG],
                                xb[im, j][0:ONES_P, :],
                            )
    nc.compile()
    return nc


_CACHE = {}


def _get_program(Wc, bc):
    key = (Wc.tobytes(), bc.tobytes())
    if key not in _CACHE:
        wqarr = _build_wq(Wc.astype(np.float64), bc.astype(np.float64))
        nc = _build_nc()
        _CACHE[key] = (nc, wqarr, _build_aux())
    return _CACHE[key]


def _install_trace_shim():
    """The agent image lacks antenv.axon_hooks; rebuild the NTFF hook from
    trn_boot's ctypes recipe and skip the artifact upload."""
    import types

    if "antenv.axon_hooks" in sys.modules:
        return
    try:
        from trn_agent_boot.trn_boot import _ntff_profile_via_ctypes

        hook = _ntff_profile_via_ctypes("/opt/axon/libaxon_pjrt.so")
    except Exception:
        hook = None
    mod = types.ModuleType("antenv.axon_hooks")
    mod.get_axon_ntff_profile_hook = lambda: hook
    mod.set_axon_ntff_profile_hook = lambda h: None
    sys.modules["antenv.axon_hooks"] = mod
    import concourse.bass_utils as bu

    bu.upload_artifacts = lambda d: "local://skipped"


def kernel(x, W, b, _trace=False, _tracedir=None):
    x = np.asarray(x)
    W = np.asarray(W)
    b = np.asarray(b)
    nc, wqarr, auxarr = _get_program(W, b)
    in_maps = []
    for c in range(NCORES):
        shard = np.ascontiguousarray(x[c * IMGS : (c + 1) * IMGS, 0]).astype(np.float32)
        in_maps.append({"xs": shard, "wq": wqarr, "aux": auxarr})
    kw = {}
    if _trace:
        _install_trace_shim()
        kw = {"trace": True, "tmpdir": _tracedir}
    res = run_bass_kernel_spmd(nc, in_maps, list(range(NCORES)), **kw)
    out = np.concatenate([res.results[c]["out"] for c in range(NCORES)], axis=0)
    out = out[:, None].astype(x.dtype)
    kernel._last = res
    return out


# revision 15
# speedup vs baseline: 1.0953x; 1.0953x over previous
"""Trainium2 Bass kernel for nn_Deep_AD_F_58213986730479 (dense_cnn).

Math (per iteration t of 3):
    feats = 4 one-pixel zero-padded shifts (N,S,W,E) of x        [n,4,h,w]
    d     = conv3x3(feats, W[t]) + b[t]                          [n,4,h,w]
    x    -= sum_k d_k * exp(-d_k^2) / 4

Implementation (v3, fp8 DoubleRow supersteps):
  - Pure data parallel: batch 32 -> 8 cores x 4 images.
  - The shift+conv composes into a 21-tap stencil: 5 column-banded matrices
    B_Dx (Dx=-2..2) applied at column shifts. On TRN2 the PE streams one
    OUTPUT column per cycle regardless of dtype, so fp8 DoubleRow (two
    (lhsT, shifted-rhs) band applications summed per instruction) is the
    only instruction-count lever: pairs (-2,-1), (0,+1), (-1res, ... ) wait
    (B-1_residual, B0) cover the 5 bands + one fp8 residual in 3 matmuls
    per (k, tile, image). Guard columns on the fp8 x copy let every pair
    run the full 512-col output range.
  - Supersteps (t, j): the 3 pair-lhsT per k are applied to all 4 images
    back-to-back (weight reuse), accumulating d_k [128, 4*512] in PSUM
    (4 banks, double-buffered per k).
  - Per-channel bias rides the Derivative_Erf activation (bias operand),
    one batched [128, 2048] ACT per (k): e = 2/sqrt(pi) exp(-(d+b)^2).
  - gated g_k = d_k * e_k on DVE (PSUM-fp32 read, bf16 out).
  - Sum tree + update in bf16 SBUF where plain TensorTensor has a fast
    2-byte path (~0.65 ns/col): s01/s23 on GpSimd, stot on DVE, update as
    (stot * masktile) then add-to-x, split DVE/GpSimd.
  - x lives in bf16 between iterations; Scalar converts it to fp8 for the
    next iteration's matmuls; the last iteration's update writes f32 into
    the staging tile which DMAs out.
  - 512 rows -> 5 row-tiles owning 103/103/103/103/100 rows with a 6-row
    halo each side (the valid region shrinks 2 rows/iter, so no cross-tile
    traffic); row-edge boundary fixes are baked into top/bot variants of
    B_{-1,0,1}; column-edge phantom corrections are 1-col fp8 matmuls.
"""
import sys

sys.path.insert(0, "/opt/trn_rl_repo")

import math
import numpy as np

import concourse.bass as bass
import concourse.bacc as bacc
import concourse.mybir as mybir
from concourse.ap import AP
from concourse.tile import TileContext
from concourse.bass_utils import run_bass_kernel_spmd

F32 = mybir.dt.float32
BF16 = mybir.dt.bfloat16
FP8 = mybir.dt.float8e4
AF = mybir.ActivationFunctionType
ALU = mybir.AluOpType
DR = mybir.MatmulPerfMode.DoubleRow

NCORES = 8
IMGS = 4
H = W_IMG = 512
T_ITERS = 3
KCH = 4
NTILES = 5
OWN = [103, 103, 103, 103, 100]      # owned rows per tile
OSTART = [0, 103, 206, 309, 412]     # first owned image row
RSTART = [-6, 97, 200, 303, 406]     # image row at partition 0
PLO = [6, 0, 0, 0, 0]                # first loaded partition
PHI = [128, 128, 128, 128, 106]      # end of loaded partitions
MLO = [6, 0, 0, 0, 0]                # update-mask range
MHI = [115, 115, 115, 115, 106]
OWN_P0 = 6                           # owned rows start here in every tile
C_UPD = math.sqrt(math.pi) / 8.0     # 1/4 * sqrt(pi)/2 (Derivative_Erf scale)
XQW = 520                            # fp8 x tile: 2 guard + 512 + guards
GL = 2                               # left guard cols
IMW = IMGS * W_IMG                   # 2048

# feats channel order in reference: N, S, W, E
OY = [-1, 1, 0, 0]
OX = [0, 0, -1, 1]

# weight tile layout per (t,k): 3 classes x 3 pairs x 256 + corrL/R = 2560
WTILE_COLS = 2560
# pair p rhs left-slot Dx (slots are (a, a+1))
PAIR_A = [-2, 1, -1]


def _composite_taps(Wc):
    taps = np.zeros((T_ITERS, KCH, 5, 5), np.float64)
    for t in range(T_ITERS):
        for k in range(KCH):
            for i in range(4):
                for dy in (-1, 0, 1):
                    for dx in (-1, 0, 1):
                        taps[t, k, dy + OY[i] + 2, dx + OX[i] + 2] += Wc[
                            t, k, i, dy + 1, dx + 1
                        ]
    return taps


def _band(vals_by_dy):
    B = np.zeros((128, 128), np.float64)
    for dy, v in vals_by_dy.items():
        B += v * np.eye(128, k=-dy)
    return B


def _build_wq(Wc):
    """fp8 weight image [128, 12*WTILE_COLS].

    Per (t,k), per class (top/mid/bot), three DoubleRow pairs; pair p's rhs
    slots read x at column shifts (a, a+1) with a = PAIR_A[p]:
      P1 a=-2: (B_{-2}, B_{-1})
      P2 a=+1: (B_{+1}, B_{+2})
      P3 a=-1: (fp8-residual of B_{-1}, B_0)
    then corrL [128], corrR [128] column-edge corrections.
    """
    taps = _composite_taps(Wc)
    npdt = mybir.dt.np(FP8)

    def q8(M):
        return M.astype(np.float32).astype(npdt).astype(np.float64)

    out = np.zeros((128, T_ITERS * KCH * WTILE_COLS), np.float64)
    for t in range(T_ITERS):
        for k in range(KCH):
            base = (t * KCH + k) * WTILE_COLS
            per_dx = {
                Dx: _band({Dy: taps[t, k, Dy + 2, Dx + 2] for Dy in range(-2, 3)})
                for Dx in (-2, -1, 0, 1, 2)
            }
            for ci, fix in enumerate((0, None, 1)):
                v = {}
                for Dx in (-2, -1, 0, 1, 2):
                    B = per_dx[Dx].copy()
                    if Dx in (-1, 0, 1):
                        if fix == 0:
                            B[OWN_P0, OWN_P0] -= Wc[t, k, 1, 0, Dx + 1]
                        elif fix == 1:
                            B[105, 105] -= Wc[t, k, 0, 2, Dx + 1]
                    v[Dx] = B
                r1 = v[-1] - q8(v[-1])  # fp8 residual of B_{-1}
                pairs = [
                    (v[-2], v[-1]),
                    (v[1], v[2]),
                    (r1, v[0]),
                ]
                for pi, (a_m, b_m) in enumerate(pairs):
                    off = base + (ci * 3 + pi) * 256
                    out[:, off : off + 128] = a_m
                    out[:, off + 128 : off + 256] = b_m
            out[:, base + 2304 : base + 2432] = _band(
                {dy: -Wc[t, k, 3, dy + 1, 0] for dy in (-1, 0, 1)}
            )
            out[:, base + 2432 : base + 2560] = _band(
                {dy: -Wc[t, k, 2, dy + 1, 2] for dy in (-1, 0, 1)}
            )
    return np.ascontiguousarray(out.astype(np.float32)).astype(npdt)


def _build_masks():
    import ml_dtypes

    m = np.zeros((128, NTILES * W_IMG), np.float32)
    for j in range(NTILES):
        m[MLO[j] : MHI[j], j * W_IMG : (j + 1) * W_IMG] = -C_UPD
    return m.astype(ml_dtypes.bfloat16)


def _build_nc(bvals):
    nc = bacc.Bacc(None, target_bir_lowering=False)
    xs = nc.declare_dram_parameter("xs", [IMGS, H, W_IMG], F32, isOutput=False)
    wq = nc.declare_dram_parameter(
        "wq", [128, T_ITERS * KCH * WTILE_COLS], FP8, isOutput=False
    )
    mk = nc.declare_dram_parameter("mk", [128, NTILES * W_IMG], BF16, isOutput=False)
    yo = nc.declare_dram_parameter("out", [IMGS, H, W_IMG], F32, isOutput=True)

    with TileContext(nc) as tc:
        with (
            tc.tile_pool(name="wts", bufs=1) as wp,
            tc.tile_pool(name="xdata", bufs=1) as xp,
            tc.tile_pool(name="work", bufs=2) as sp,
            tc.tile_pool(name="ps", bufs=2, space="PSUM") as pp,
        ):
            maskt = wp.tile([128, NTILES * W_IMG], BF16, tag="maskt")
            nc.sync.dma_start(out=maskt[:], in_=mk[:])
            bias_t = wp.tile([128, T_ITERS * KCH], F32, tag="bias")

            wt = {}
            for t in range(T_ITERS):
                for k in range(KCH):
                    wt[t, k] = wp.tile(
                        [128, WTILE_COLS], FP8, tag=f"wt{t}_{k}", name=f"wt{t}_{k}"
                    )

            def load_wt(t, k):
                off = (t * KCH + k) * WTILE_COLS
                nc.sync.dma_start(out=wt[t, k][:], in_=wq[:, off : off + WTILE_COLS])

            stage, xb, xq = {}, {}, {}
            conv_eng = [nc.scalar, nc.gpsimd, nc.vector]
            for im in range(IMGS):
                for j in range(NTILES):
                    st = xp.tile(
                        [128, W_IMG], F32, tag=f"st{im}_{j}", name=f"st{im}_{j}"
                    )
                    stage[im, j] = st
                    if PLO[j] > 0:
                        nc.vector.memset(st[0 : PLO[j], :], 0.0)
                    if PHI[j] < 128:
                        p0 = (PHI[j] // 32) * 32
                        nc.vector.memset(st[p0:128, :], 0.0)
                    nc.sync.dma_start(
                        out=st[PLO[j] : PHI[j], :],
                        in_=xs[im, RSTART[j] + PLO[j] : RSTART[j] + PHI[j], :],
                    )
                    if im == 0 and j < KCH:
                        load_wt(0, j)
                if 1 <= im < T_ITERS:
                    for k in range(KCH):
                        load_wt(im, k)
                if im == 0:
                    # emitted after image-0 loads so the memsets don't gate
                    # the first x DMA on the vector queue
                    for t in range(T_ITERS):
                        for k in range(KCH):
                            c = t * KCH + k
                            nc.vector.memset(
                                bias_t[:, c : c + 1], float(bvals[t, k])
                            )

            for im in range(IMGS):
                for j in range(NTILES):
                    q = xp.tile([128, XQW], FP8, tag=f"xq{im}_{j}", name=f"xq{im}_{j}")
                    xq[im, j] = q
                    nc.vector.memset(q[:], 0.0)
                    xb[im, j] = xp.tile(
                        [128, W_IMG], BF16, tag=f"xb{im}_{j}", name=f"xb{im}_{j}"
                    )
                    eng = conv_eng[(im * NTILES + j) % 3]
                    if eng is nc.scalar:
                        eng.copy(q[:, GL : GL + W_IMG], stage[im, j][:])
                    else:
                        eng.tensor_copy(
                            out=q[:, GL : GL + W_IMG], in_=stage[im, j][:]
                        )

            lhs_dims = [[WTILE_COLS, 128], [128, 2], [1, 128]]

            for t in range(T_ITERS):
                for j in range(NTILES):
                    cls = 0 if j == 0 else (2 if j == NTILES - 1 else 1)
                    gk = []
                    for k in range(KCH):
                        w = wt[t, k]
                        wh, woff = w[:].tensor, w[:].offset
                        d = pp.tile([128, IMW], F32, tag="d", name="d")
                        for pi in range(3):
                            wcol = (cls * 3 + pi) * 256
                            lhs = AP(wh, woff + wcol, lhs_dims)
                            a = PAIR_A[pi]
                            for im in range(IMGS):
                                q = xq[im, j]
                                qh, qoff = q[:].tensor, q[:].offset
                                rhs = AP(
                                    qh,
                                    qoff + GL + a,
                                    [[XQW, 128], [1, 2], [1, W_IMG]],
                                )
                                nc.tensor.matmul(
                                    d[:, im * W_IMG : (im + 1) * W_IMG],
                                    lhs,
                                    rhs,
                                    start=(pi == 0),
                                    stop=False,
                                    perf_mode=DR,
                                )
                        for im in range(IMGS):
                            q = xq[im, j]
                            nc.tensor.matmul(
                                d[:, im * W_IMG : im * W_IMG + 1],
                                w[:, 2304:2432],
                                q[:, GL : GL + 1],
                                start=False,
                                stop=False,
                            )
                            nc.tensor.matmul(
                                d[:, (im + 1) * W_IMG - 1 : (im + 1) * W_IMG],
                                w[:, 2432:2560],
                                q[:, GL + W_IMG - 1 : GL + W_IMG],
                                start=False,
                                stop=(im == IMGS - 1),
                            )
                        e = sp.tile([128, IMW], BF16, tag=f"e{k % 2}")
                        c = t * KCH + k
                        nc.scalar.activation(
                            e[:], d[:], AF.Derivative_Erf,
                            bias=bias_t[:, c : c + 1],
                        )
                        g = sp.tile([128, IMW], BF16, tag=f"g{k}")
                        nc.vector.scalar_tensor_tensor(
                            out=g[:],
                            in0=d[:],
                            scalar=bias_t[:, c : c + 1],
                            in1=e[:],
                            op0=ALU.add,
                            op1=ALU.mult,
                        )
                        gk.append(g)
                    s01 = sp.tile([128, IMW], BF16, tag="s01")
                    nc.gpsimd.tensor_tensor(
                        out=s01[:], in0=gk[0][:], in1=gk[1][:], op=ALU.add
                    )
                    s23 = sp.tile([128, IMW], BF16, tag="s23")
                    nc.gpsimd.tensor_tensor(
                        out=s23[:], in0=gk[2][:], in1=gk[3][:], op=ALU.add
                    )
                    stot = sp.tile([128, IMW], BF16, tag="stot")
                    nc.vector.tensor_tensor(
                        out=stot[:], in0=s01[:], in1=s23[:], op=ALU.add
                    )
                    mask = maskt[:, j * W_IMG : (j + 1) * W_IMG]
                    for im in range(IMGS):
                        blk = stot[:, im * W_IMG : (im + 1) * W_IMG]
                        tmp = sp.tile([128, W_IMG], BF16, tag="tmp")
                        teng = nc.vector if im % 2 == 0 else nc.gpsimd
                        teng.tensor_tensor(
                            out=tmp[:], in0=blk, in1=mask, op=ALU.mult
                        )
                        if t == 0:
                            nc.vector.tensor_tensor(
                                out=xb[im, j][:],
                                in0=tmp[:],
                                in1=stage[im, j][:],
                                op=ALU.add,
                            )
                        elif t == T_ITERS - 1:
                            nc.vector.tensor_tensor(
                                out=stage[im, j][:],
                                in0=tmp[:],
                                in1=xb[im, j][:],
                                op=ALU.add,
                            )
                            nc.sync.dma_start(
                                out=yo[im, OSTART[j] : OSTART[j] + OWN[j], :],
                                in_=stage[im, j][OWN_P0 : OWN_P0 + OWN[j], :],
                            )
                        else:
                            nc.vector.tensor_tensor(
                                out=xb[im, j][:],
                                in0=tmp[:],
                                in1=xb[im, j][:],
                                op=ALU.add,
                            )
                        if t < T_ITERS - 1:
                            nc.scalar.copy(
                                xq[im, j][:, GL : GL + W_IMG], xb[im, j][:]
                            )
    nc.compile()
    return nc


_CACHE = {}


def _get_program(Wc, bc):
    key = (Wc.tobytes(), bc.tobytes())
    if key not in _CACHE:
        wqarr = _build_wq(Wc.astype(np.float64))
        nc = _build_nc(bc.astype(np.float64))
        _CACHE[key] = (nc, wqarr, _build_masks())
    return _CACHE[key]


def _install_trace_shim():
    """The agent image lacks antenv.axon_hooks; rebuild the NTFF hook from
    trn_boot's ctypes recipe and skip the artifact upload."""
    import types

    if "antenv.axon_hooks" in sys.modules:
        return
    try:
        from trn_agent_boot.trn_boot import _ntff_profile_via_ctypes

        hook = _ntff_profile_via_ctypes("/opt/axon/libaxon_pjrt.so")
    except Exception:
        hook = None
    mod = types.ModuleType("antenv.axon_hooks")
    mod.get_axon_ntff_profile_hook = lambda: hook
    mod.set_axon_ntff_profile_hook = lambda h: None
    sys.modules["antenv.axon_hooks"] = mod
    import concourse.bass_utils as bu

    bu.upload_artifacts = lambda d: "local://skipped"


def kernel(x, W, b, _trace=False, _tracedir=None):
    x = np.asarray(x)
    W = np.asarray(W)
    b = np.asarray(b)
    nc, wqarr, mkarr = _get_program(W, b)
    in_maps = []
    for c in range(NCORES):
        shard = np.ascontiguousarray(x[c * IMGS : (c + 1) * IMGS, 0]).astype(np.float32)
        in_maps.append({"xs": shard, "wq": wqarr, "mk": mkarr})
    kw = {}
    if _trace:
        _install_trace_shim()
        kw = {"trace": True, "tmpdir": _tracedir}
    res = run_bass_kernel_spmd(nc, in_maps, list(range(NCORES)), **kw)
    out = np.concatenate([res.results[c]["out"] for c in range(NCORES)], axis=0)
    out = out[:, None].astype(x.dtype)
    kernel._last = res
    return out


# revision 16
# speedup vs baseline: 1.1498x; 1.0497x over previous
"""Trainium2 Bass kernel for nn_Deep_AD_F_58213986730479 (dense_cnn).

Math (per iteration t of 3):
    feats = 4 one-pixel zero-padded shifts (N,S,W,E) of x        [n,4,h,w]
    d     = conv3x3(feats, W[t]) + b[t]                          [n,4,h,w]
    x    -= sum_k d_k * exp(-d_k^2) / 4

Implementation (v3, fp8 DoubleRow supersteps):
  - Pure data parallel: batch 32 -> 8 cores x 4 images.
  - The shift+conv composes into a 21-tap stencil: 5 column-banded matrices
    B_Dx (Dx=-2..2) applied at column shifts. On TRN2 the PE streams one
    OUTPUT column per cycle regardless of dtype, so fp8 DoubleRow (two
    (lhsT, shifted-rhs) band applications summed per instruction) is the
    only instruction-count lever: pairs (-2,-1), (0,+1), (-1res, ... ) wait
    (B-1_residual, B0) cover the 5 bands + one fp8 residual in 3 matmuls
    per (k, tile, image). Guard columns on the fp8 x copy let every pair
    run the full 512-col output range.
  - Supersteps (t, j): the 3 pair-lhsT per k are applied to all 4 images
    back-to-back (weight reuse), accumulating d_k [128, 4*512] in PSUM
    (4 banks, double-buffered per k).
  - Per-channel bias rides the Derivative_Erf activation (bias operand),
    one batched [128, 2048] ACT per (k): e = 2/sqrt(pi) exp(-(d+b)^2).
  - gated g_k = d_k * e_k on DVE (PSUM-fp32 read, bf16 out).
  - Sum tree + update in bf16 SBUF where plain TensorTensor has a fast
    2-byte path (~0.65 ns/col): s01/s23 on GpSimd, stot on DVE, update as
    (stot * masktile) then add-to-x, split DVE/GpSimd.
  - x lives in bf16 between iterations; Scalar converts it to fp8 for the
    next iteration's matmuls; the last iteration's update writes f32 into
    the staging tile which DMAs out.
  - 512 rows -> 5 row-tiles owning 103/103/103/103/100 rows with a 6-row
    halo each side (the valid region shrinks 2 rows/iter, so no cross-tile
    traffic); row-edge boundary fixes are baked into top/bot variants of
    B_{-1,0,1}; column-edge phantom corrections are 1-col fp8 matmuls.
"""
import sys

sys.path.insert(0, "/opt/trn_rl_repo")

import math
import numpy as np

import concourse.bass as bass
import concourse.bacc as bacc
import concourse.mybir as mybir
from concourse.ap import AP
from concourse.tile import TileContext
from concourse.bass_utils import run_bass_kernel_spmd

F32 = mybir.dt.float32
BF16 = mybir.dt.bfloat16
FP8 = mybir.dt.float8e4
AF = mybir.ActivationFunctionType
ALU = mybir.AluOpType
DR = mybir.MatmulPerfMode.DoubleRow

NCORES = 8
IMGS = 4
H = W_IMG = 512
T_ITERS = 3
KCH = 4
NTILES = 5
OWN = [103, 103, 103, 103, 100]      # owned rows per tile
OSTART = [0, 103, 206, 309, 412]     # first owned image row
RSTART = [-6, 97, 200, 303, 406]     # image row at partition 0
PLO = [6, 0, 0, 0, 0]                # first loaded partition
PHI = [128, 128, 128, 128, 106]      # end of loaded partitions
MLO = [6, 0, 0, 0, 0]                # update-mask range
MHI = [115, 115, 115, 115, 106]
OWN_P0 = 6                           # owned rows start here in every tile
C_UPD = math.sqrt(math.pi) / 8.0     # 1/4 * sqrt(pi)/2 (Derivative_Erf scale)
XQW = 520                            # fp8 x tile: 2 guard + 512 + guards
GL = 2                               # left guard cols
IMW = IMGS * W_IMG                   # 2048

# feats channel order in reference: N, S, W, E
OY = [-1, 1, 0, 0]
OX = [0, 0, -1, 1]

# weight tile layout per (t,k): 3 classes x 3 pairs x 256 + corrL/R = 2560
WTILE_COLS = 2560
# pair p rhs left-slot Dx (slots are (a, a+1))
PAIR_A = [-2, 1, -1]


def _composite_taps(Wc):
    taps = np.zeros((T_ITERS, KCH, 5, 5), np.float64)
    for t in range(T_ITERS):
        for k in range(KCH):
            for i in range(4):
                for dy in (-1, 0, 1):
                    for dx in (-1, 0, 1):
                        taps[t, k, dy + OY[i] + 2, dx + OX[i] + 2] += Wc[
                            t, k, i, dy + 1, dx + 1
                        ]
    return taps


def _band(vals_by_dy):
    B = np.zeros((128, 128), np.float64)
    for dy, v in vals_by_dy.items():
        B += v * np.eye(128, k=-dy)
    return B


def _build_wq(Wc):
    """fp8 weight image [128, 12*WTILE_COLS].

    Per (t,k), per class (top/mid/bot), three DoubleRow pairs; pair p's rhs
    slots read x at column shifts (a, a+1) with a = PAIR_A[p]:
      P1 a=-2: (B_{-2}, B_{-1})
      P2 a=+1: (B_{+1}, B_{+2})
      P3 a=-1: (fp8-residual of B_{-1}, B_0)
    then corrL [128], corrR [128] column-edge corrections.
    """
    taps = _composite_taps(Wc)
    npdt = mybir.dt.np(FP8)

    def q8(M):
        return M.astype(np.float32).astype(npdt).astype(np.float64)

    out = np.zeros((128, T_ITERS * KCH * WTILE_COLS), np.float64)
    for t in range(T_ITERS):
        for k in range(KCH):
            base = (t * KCH + k) * WTILE_COLS
            per_dx = {
                Dx: _band({Dy: taps[t, k, Dy + 2, Dx + 2] for Dy in range(-2, 3)})
                for Dx in (-2, -1, 0, 1, 2)
            }
            for ci, fix in enumerate((0, None, 1)):
                v = {}
                for Dx in (-2, -1, 0, 1, 2):
                    B = per_dx[Dx].copy()
                    if Dx in (-1, 0, 1):
                        if fix == 0:
                            B[OWN_P0, OWN_P0] -= Wc[t, k, 1, 0, Dx + 1]
                        elif fix == 1:
                            B[105, 105] -= Wc[t, k, 0, 2, Dx + 1]
                    v[Dx] = B
                r1 = v[-1] - q8(v[-1])  # fp8 residual of B_{-1}
                pairs = [
                    (v[-2], v[-1]),
                    (v[1], v[2]),
                    (r1, v[0]),
                ]
                for pi, (a_m, b_m) in enumerate(pairs):
                    off = base + (ci * 3 + pi) * 256
                    out[:, off : off + 128] = a_m
                    out[:, off + 128 : off + 256] = b_m
            out[:, base + 2304 : base + 2432] = _band(
                {dy: -Wc[t, k, 3, dy + 1, 0] for dy in (-1, 0, 1)}
            )
            out[:, base + 2432 : base + 2560] = _band(
                {dy: -Wc[t, k, 2, dy + 1, 2] for dy in (-1, 0, 1)}
            )
    return np.ascontiguousarray(out.astype(np.float32)).astype(npdt)


def _build_masks():
    import ml_dtypes

    m = np.zeros((128, NTILES * W_IMG), np.float32)
    for j in range(NTILES):
        m[MLO[j] : MHI[j], j * W_IMG : (j + 1) * W_IMG] = -C_UPD
    return m.astype(ml_dtypes.bfloat16)


def _build_nc(bvals):
    nc = bacc.Bacc(None, target_bir_lowering=False)
    xs = nc.declare_dram_parameter("xs", [IMGS, H, W_IMG], F32, isOutput=False)
    wq = nc.declare_dram_parameter(
        "wq", [128, T_ITERS * KCH * WTILE_COLS], FP8, isOutput=False
    )
    mk = nc.declare_dram_parameter("mk", [128, NTILES * W_IMG], BF16, isOutput=False)
    yo = nc.declare_dram_parameter("out", [IMGS, H, W_IMG], F32, isOutput=True)

    with TileContext(nc) as tc:
        with (
            tc.tile_pool(name="wts", bufs=1) as wp,
            tc.tile_pool(name="xdata", bufs=1) as xp,
            tc.tile_pool(name="work", bufs=2) as sp,
            tc.tile_pool(name="ps", bufs=2, space="PSUM") as pp,
        ):
            maskt = wp.tile([128, NTILES * W_IMG], BF16, tag="maskt")
            nc.sync.dma_start(out=maskt[:], in_=mk[:])
            bias_t = wp.tile([128, T_ITERS * KCH], F32, tag="bias")

            wt = {}
            for t in range(T_ITERS):
                for k in range(KCH):
                    wt[t, k] = wp.tile(
                        [128, WTILE_COLS], FP8, tag=f"wt{t}_{k}", name=f"wt{t}_{k}"
                    )

            def load_wt(t, k):
                off = (t * KCH + k) * WTILE_COLS
                nc.sync.dma_start(out=wt[t, k][:], in_=wq[:, off : off + WTILE_COLS])

            stage, xq = {}, {}
            xb = ({}, {})
            conv_eng = [nc.scalar, nc.gpsimd, nc.vector]
            for im in range(IMGS):
                for j in range(NTILES):
                    st = xp.tile(
                        [128, W_IMG], F32, tag=f"st{im}_{j}", name=f"st{im}_{j}"
                    )
                    stage[im, j] = st
                    if PLO[j] > 0:
                        nc.vector.memset(st[0 : PLO[j], :], 0.0)
                    if PHI[j] < 128:
                        p0 = (PHI[j] // 32) * 32
                        nc.vector.memset(st[p0:128, :], 0.0)
                    nc.sync.dma_start(
                        out=st[PLO[j] : PHI[j], :],
                        in_=xs[im, RSTART[j] + PLO[j] : RSTART[j] + PHI[j], :],
                    )
                    if im == 0 and j < KCH:
                        load_wt(0, j)
                if 1 <= im < T_ITERS:
                    for k in range(KCH):
                        load_wt(im, k)
                if im == 0:
                    # emitted after image-0 loads so the memsets don't gate
                    # the first x DMA on the vector queue
                    for t in range(T_ITERS):
                        for k in range(KCH):
                            c = t * KCH + k
                            nc.vector.memset(
                                bias_t[:, c : c + 1], float(bvals[t, k])
                            )

            for im in range(IMGS):
                for j in range(NTILES):
                    q = xp.tile([128, XQW], FP8, tag=f"xq{im}_{j}", name=f"xq{im}_{j}")
                    xq[im, j] = q
                    nc.vector.memset(q[:], 0.0)
                    for v in range(2):
                        xb[v][im, j] = xp.tile(
                            [128, W_IMG], BF16,
                            tag=f"xb{v}_{im}_{j}", name=f"xb{v}_{im}_{j}",
                        )
                    eng = conv_eng[(im * NTILES + j) % 3]
                    if eng is nc.scalar:
                        eng.copy(q[:, GL : GL + W_IMG], stage[im, j][:])
                    else:
                        eng.tensor_copy(
                            out=q[:, GL : GL + W_IMG], in_=stage[im, j][:]
                        )

            lhs_dims = [[WTILE_COLS, 128], [128, 2], [1, 128]]
            HW2 = 2 * W_IMG  # half-superstep width (2 images)

            for t in range(T_ITERS):
                for j in range(NTILES):
                    cls = 0 if j == 0 else (2 if j == NTILES - 1 else 1)
                    gk = []
                    for k in range(KCH):
                        c = t * KCH + k
                        w = wt[t, k]
                        wh, woff = w[:].tensor, w[:].offset
                        g = sp.tile([128, IMW], BF16, tag=f"g{k}")
                        # two PSUM halves (2 images each) so ACT/gated can
                        # drain half k while the PE fills the other half
                        for h in range(2):
                            d = pp.tile([128, HW2], F32, tag=f"d{h}", name=f"d{h}")
                            for pi in range(3):
                                wcol = (cls * 3 + pi) * 256
                                lhs = AP(wh, woff + wcol, lhs_dims)
                                a = PAIR_A[pi]
                                for hi, im in enumerate((2 * h, 2 * h + 1)):
                                    q = xq[im, j]
                                    qh, qoff = q[:].tensor, q[:].offset
                                    rhs = AP(
                                        qh,
                                        qoff + GL + a,
                                        [[XQW, 128], [1, 2], [1, W_IMG]],
                                    )
                                    nc.tensor.matmul(
                                        d[:, hi * W_IMG : (hi + 1) * W_IMG],
                                        lhs,
                                        rhs,
                                        start=(pi == 0),
                                        stop=False,
                                        perf_mode=DR,
                                    )
                            for hi, im in enumerate((2 * h, 2 * h + 1)):
                                q = xq[im, j]
                                nc.tensor.matmul(
                                    d[:, hi * W_IMG : hi * W_IMG + 1],
                                    w[:, 2304:2432],
                                    q[:, GL : GL + 1],
                                    start=False,
                                    stop=False,
                                )
                                nc.tensor.matmul(
                                    d[:, (hi + 1) * W_IMG - 1 : (hi + 1) * W_IMG],
                                    w[:, 2432:2560],
                                    q[:, GL + W_IMG - 1 : GL + W_IMG],
                                    start=False,
                                    stop=(hi == 1),
                                )
                            e = sp.tile([128, HW2], BF16, tag=f"e{h}")
                            nc.scalar.activation(
                                e[:], d[:], AF.Derivative_Erf,
                                bias=bias_t[:, c : c + 1],
                            )
                            nc.vector.scalar_tensor_tensor(
                                out=g[:, h * HW2 : (h + 1) * HW2],
                                in0=d[:],
                                scalar=bias_t[:, c : c + 1],
                                in1=e[:],
                                op0=ALU.add,
                                op1=ALU.mult,
                            )
                        gk.append(g)
                    s01 = sp.tile([128, IMW], BF16, tag="s01")
                    nc.gpsimd.tensor_tensor(
                        out=s01[:], in0=gk[0][:], in1=gk[1][:], op=ALU.add
                    )
                    s23 = sp.tile([128, IMW], BF16, tag="s23")
                    nc.gpsimd.tensor_tensor(
                        out=s23[:], in0=gk[2][:], in1=gk[3][:], op=ALU.add
                    )
                    stot = sp.tile([128, IMW], BF16, tag="stot")
                    nc.vector.tensor_tensor(
                        out=stot[:], in0=s01[:], in1=s23[:], op=ALU.add
                    )
                    mask = maskt[:, j * W_IMG : (j + 1) * W_IMG]
                    tmps = []
                    for im in range(IMGS):
                        blk = stot[:, im * W_IMG : (im + 1) * W_IMG]
                        tmp = sp.tile([128, W_IMG], BF16, tag=f"tmp{im % 2}")
                        nc.gpsimd.tensor_tensor(
                            out=tmp[:], in0=blk, in1=mask, op=ALU.mult
                        )
                        tmps.append(tmp)
                    for im in range(IMGS):
                        tmp = tmps[im]
                        if t == 0:
                            nc.vector.tensor_tensor(
                                out=xb[0][im, j][:],
                                in0=tmp[:],
                                in1=stage[im, j][:],
                                op=ALU.add,
                            )
                        elif t == T_ITERS - 1:
                            nc.vector.tensor_tensor(
                                out=stage[im, j][:],
                                in0=tmp[:],
                                in1=xb[1][im, j][:],
                                op=ALU.add,
                            )
                            nc.sync.dma_start(
                                out=yo[im, OSTART[j] : OSTART[j] + OWN[j], :],
                                in_=stage[im, j][OWN_P0 : OWN_P0 + OWN[j], :],
                            )
                        else:
                            nc.vector.tensor_tensor(
                                out=xb[1][im, j][:],
                                in0=tmp[:],
                                in1=xb[0][im, j][:],
                                op=ALU.add,
                            )
                        if t < T_ITERS - 1:
                            nc.scalar.copy(
                                xq[im, j][:, GL : GL + W_IMG], xb[t][im, j][:]
                            )
    nc.compile()
    return nc


_CACHE = {}


def _get_program(Wc, bc):
    key = (Wc.tobytes(), bc.tobytes())
    if key not in _CACHE:
        wqarr = _build_wq(Wc.astype(np.float64))
        nc = _build_nc(bc.astype(np.float64))
        _CACHE[key] = (nc, wqarr, _build_masks())
    return _CACHE[key]


def _install_trace_shim():
    """The agent image lacks antenv.axon_hooks; rebuild the NTFF hook from
    trn_boot's ctypes recipe and skip the artifact upload."""
    import types

    if "antenv.axon_hooks" in sys.modules:
        return
    try:
        from trn_agent_boot.trn_boot import _ntff_profile_via_ctypes

        hook = _ntff_profile_via_ctypes("/opt/axon/libaxon_pjrt.so")
    except Exception:
        hook = None
    mod = types.ModuleType("antenv.axon_hooks")
    mod.get_axon_ntff_profile_hook = lambda: hook
    mod.set_axon_ntff_profile_hook = lambda h: None
    sys.modules["antenv.axon_hooks"] = mod
    import concourse.bass_utils as bu

    bu.upload_artifacts = lambda d: "local://skipped"


def kernel(x, W, b, _trace=False, _tracedir=None):
    x = np.asarray(x)
    W = np.asarray(W)
    b = np.asarray(b)
    nc, wqarr, mkarr = _get_program(W, b)
    in_maps = []
    for c in range(NCORES):
        shard = np.ascontiguousarray(x[c * IMGS : (c + 1) * IMGS, 0]).astype(np.float32)
        in_maps.append({"xs": shard, "wq": wqarr, "mk": mkarr})
    kw = {}
    if _trace:
        _install_trace_shim()
        kw = {"trace": True, "tmpdir": _tracedir}
    res = run_bass_kernel_spmd(nc, in_maps, list(range(NCORES)), **kw)
    out = np.concatenate([res.results[c]["out"] for c in range(NCORES)], axis=0)
    out = out[:, None].astype(x.dtype)
    kernel._last = res
    return out
